# revision 2
# baseline (speedup 1.0000x reference)
"""Multi-head causal attention (kqv proj + softmax(QK^T)V) on 8 TRN2 NeuronCores.

Sharding: 8 cores = 4 batches x 2 head-groups (8 heads each). Each core is
fully independent (no collectives); host shards inputs / concats outputs.

Per-core kernel (bf16 matmuls, f32 psum/output):
  phase 1: Q^T/K^T [64, T] per head (2 heads packed into 128 partitions) and
           V [T, 64] per head produced straight from the kqv matmul -- layouts
           chosen so no on-device transpose is ever needed.
  phase 2: S^T[j,i] = K^T.T @ Q^T tiles (only causal j<=i tiles), exp on
           ScalarE with the 1/8 scale fused into the activation, 0/1 mask
           multiply on diagonal tiles only, then out[i,:] = (E^T.T @ [V|1])
           normalized by the appended denominator column + v-bias.
"""

import sys

if "/opt/trn_rl_repo" not in sys.path:
    sys.path.insert(0, "/opt/trn_rl_repo")

import numpy as np
import ml_dtypes

DIM = 1024
NUM_HEADS = 16
SEQ = 2048
BATCH = 4
D = 64  # head dim
SCALE = D**-0.5
N_CORES = 8
HPC = 8  # heads per core
PAIRS = HPC // 2
CC = DIM // 128  # contraction chunks (8)
TCH = SEQ // 512  # 512-wide token chunks (4)
TT = SEQ // 128  # 128-wide token tiles (16)

BF16 = ml_dtypes.bfloat16

_CACHE = {}


def _build_nc():
    import concourse.tile as tile
    from concourse import bacc, mybir

    bf = mybir.dt.bfloat16
    f32 = mybir.dt.float32

    nc = bacc.Bacc("TRN2", target_bir_lowering=False)

    xT_d = nc.declare_dram_parameter("xT", [DIM, SEQ], bf, isOutput=False)
    wT_d = nc.declare_dram_parameter("wT", [DIM, 3 * 512], bf, isOutput=False)
    bqk_d = nc.declare_dram_parameter("bqk", [128, 2 * PAIRS], f32, isOutput=False)
    bv_d = nc.declare_dram_parameter("bv", [128, 512], f32, isOutput=False)
    out_d = nc.declare_dram_parameter("out", [SEQ, 512], f32, isOutput=True)

    with tile.TileContext(nc) as tc:
        with (
            tc.tile_pool(name="persist", bufs=1) as persist,
            tc.tile_pool(name="epool", bufs=20) as epool,
            tc.tile_pool(name="ost", bufs=8) as ost,
            tc.tile_pool(name="rpool", bufs=8) as rpool,
            tc.tile_pool(name="pbig", bufs=4, space="PSUM") as pbig,
            tc.tile_pool(name="ppv", bufs=3, space="PSUM") as ppv,
        ):
            # ---- constants ----
            zb = persist.tile([128, 1], f32, tag="zb")
            nc.vector.memset(zb, 0.0)

            bqk_sb = persist.tile([128, 2 * PAIRS], f32, tag="bqk")
            nc.sync.dma_start(out=bqk_sb, in_=bqk_d[:])
            bv_sb = persist.tile([128, 512], f32, tag="bv")
            nc.sync.dma_start(out=bv_sb, in_=bv_d[:])

            # causal 0/1 masks for the 4 diagonal-crossing tile offsets:
            # mask[r][jj, ii] = 1 if ii >= jj + 128*r else 0
            masks = []
            for r in range(4):
                m = persist.tile([128, 512], bf, tag=f"mask{r}")
                nc.gpsimd.memset(m, 1.0)
                nc.gpsimd.affine_select(
                    out=m,
                    in_=m,
                    compare_op=mybir.AluOpType.is_ge,
                    fill=0.0,
                    base=-128 * r,
                    pattern=[[1, 512]],
                    channel_multiplier=-1,
                )
                masks.append(m)

            # ---- inputs ----
            xT = []
            for c in range(CC):
                t = persist.tile([128, SEQ], bf, tag=f"xT{c}")
                nc.sync.dma_start(out=t, in_=xT_d[c * 128 : (c + 1) * 128, :])
                xT.append(t)
            wT = []
            for c in range(CC):
                t = persist.tile([128, 3 * 512], bf, tag=f"wT{c}")
                nc.sync.dma_start(out=t, in_=wT_d[c * 128 : (c + 1) * 128, :])
                wT.append(t)

            # ---- phase 1: projections ----
            # Q^T / K^T: per head-pair p, psum [128(2 heads x 64d), 512 t]
            QT = []
            KT = []
            for p in range(PAIRS):
                qt = persist.tile([128, SEQ], bf, tag=f"qt{p}")
                kt = persist.tile([128, SEQ], bf, tag=f"kt{p}")
                QT.append(qt)
                KT.append(kt)
            for p in range(PAIRS):
                for which, dst, bcol in ((0, QT[p], p), (1, KT[p], PAIRS + p)):
                    wcol = which * 512 + p * 128
                    for t in range(TCH):
                        ps = pbig.tile([128, 512], f32, tag="big")
                        for c in range(CC):
                            nc.tensor.matmul(
                                ps,
                                wT[c][:, wcol : wcol + 128],
                                xT[c][:, t * 512 : (t + 1) * 512],
                                start=(c == 0),
                                stop=(c == CC - 1),
                            )
                        nc.vector.tensor_scalar_add(
                            dst[:, t * 512 : (t + 1) * 512],
                            ps,
                            bqk_sb[:, bcol : bcol + 1],
                        )

            # V' tiles [128 t, 8 heads x (64 d + ones col)]
            Vp = []
            for tt in range(TT):
                vp = persist.tile([128, HPC, D + 1], bf, tag=f"vp{tt}")
                Vp.append(vp)
            for tt in range(TT):
                ps = pbig.tile([128, 512], f32, tag="big")
                for c in range(CC):
                    nc.tensor.matmul(
                        ps,
                        xT[c][:, tt * 128 : (tt + 1) * 128],
                        wT[c][:, 1024:1536],
                        start=(c == 0),
                        stop=(c == CC - 1),
                    )
                nc.vector.tensor_copy(
                    out=Vp[tt][:, :, 0:D],
                    in_=ps.rearrange("p (h d) -> p h d", h=HPC),
                )
                nc.vector.memset(Vp[tt][:, :, D : D + 1], 1.0)

            # ---- phase 2: attention ----
            for ic in range(TCH):  # 512-wide query chunk
                stage = [
                    ost.tile([128, 512], f32, tag="ostage", name=f"stage_{ic}_{i}")
                    for i in range(4)
                ]
                njt = 4 * ic + 4  # causal: j tiles 0..4ic+3
                for h in range(HPC):
                    p, po = h // 2, (h % 2) * D
                    e_tiles = []
                    for jt in range(njt):
                        ps = pbig.tile([128, 512], f32, tag="big")
                        nc.tensor.matmul(
                            ps,
                            KT[p][po : po + D, jt * 128 : (jt + 1) * 128],
                            QT[p][po : po + D, ic * 512 : (ic + 1) * 512],
                        )
                        e = epool.tile([128, 512], bf, tag="e")
                        nc.scalar.activation(
                            e,
                            ps,
                            mybir.ActivationFunctionType.Exp,
                            bias=zb,
                            scale=SCALE,
                        )
                        r = jt - 4 * ic
                        if r >= 0:
                            nc.vector.tensor_tensor(
                                e, e, masks[r], mybir.AluOpType.mult
                            )
                        e_tiles.append(e)
                    for itl in range(4):
                        it = 4 * ic + itl
                        pv = ppv.tile([128, D + 1], f32, tag="pv")
                        for jt in range(it + 1):
                            nc.tensor.matmul(
                                pv,
                                e_tiles[jt][:, itl * 128 : (itl + 1) * 128],
                                Vp[jt][:, h, :],
                                start=(jt == 0),
                                stop=(jt == it),
                            )
                        rec = rpool.tile([128, 1], f32, tag="rec")
                        nc.vector.reciprocal(rec, pv[:, D : D + 1])
                        nc.vector.scalar_tensor_tensor(
                            out=stage[itl][:, h * D : (h + 1) * D],
                            in0=pv[:, 0:D],
                            scalar=rec,
                            in1=bv_sb[:, h * D : (h + 1) * D],
                            op0=mybir.AluOpType.mult,
                            op1=mybir.AluOpType.add,
                        )
                for itl in range(4):
                    it = 4 * ic + itl
                    nc.sync.dma_start(
                        out=out_d[it * 128 : (it + 1) * 128, :], in_=stage[itl]
                    )

    nc.compile()
    return nc


def _get_nc():
    if "nc" not in _CACHE:
        _CACHE["nc"] = _build_nc()
    return _CACHE["nc"]


def _make_in_maps(x, w_kqv, b_kqv):
    """Shard: core c -> batch c//2, head-group c%2 (heads hg*8..hg*8+7)."""
    in_maps = []
    for c in range(N_CORES):
        b, hg = divmod(c, 2)
        h0 = hg * HPC
        xT = np.ascontiguousarray(x[b].T).astype(BF16)

        # weight columns, all transposed to [DIM(c), out]:
        # [q pairs (4x128) | k pairs (4x128) | v heads (512)]
        cols = []
        for which in (1, 0):  # q rows live at 1024+, k rows at 0+
            base = which * DIM
            for p in range(PAIRS):
                rows = w_kqv[base + (h0 + 2 * p) * D : base + (h0 + 2 * p + 2) * D, :]
                cols.append(rows.T)
        cols.append(w_kqv[2 * DIM + h0 * D : 2 * DIM + (h0 + HPC) * D, :].T)
        wT = np.ascontiguousarray(np.concatenate(cols, axis=1)).astype(BF16)

        bqk = np.empty((128, 2 * PAIRS), np.float32)
        for p in range(PAIRS):
            bqk[:, p] = b_kqv[DIM + (h0 + 2 * p) * D : DIM + (h0 + 2 * p + 2) * D]
            bqk[:, PAIRS + p] = b_kqv[(h0 + 2 * p) * D : (h0 + 2 * p + 2) * D]
        bv = np.tile(
            b_kqv[2 * DIM + h0 * D : 2 * DIM + (h0 + HPC) * D][None, :].astype(
                np.float32
            ),
            (128, 1),
        )
        in_maps.append({"xT": xT, "wT": wT, "bqk": bqk, "bv": bv})
    return in_maps


def run(x, w_kqv, b_kqv, trace=False, **kwargs):
    from concourse.bass_utils import run_bass_kernel_spmd

    nc = _get_nc()
    in_maps = _make_in_maps(x, w_kqv, b_kqv)
    res = run_bass_kernel_spmd(
        nc, in_maps, core_ids=list(range(N_CORES)), trace=trace, **kwargs
    )
    out = np.empty((BATCH, SEQ, DIM), np.float32)
    for c in range(N_CORES):
        b, hg = divmod(c, 2)
        out[b, :, hg * 512 : (hg + 1) * 512] = res.results[c]["out"]
    return out, res


def kernel(x, w_kqv, b_kqv):
    out, _ = run(
        np.asarray(x, np.float32),
        np.asarray(w_kqv, np.float32),
        np.asarray(b_kqv, np.float32),
    )
    return out
